# revision 42
# baseline (speedup 1.0000x reference)
"""Trainium2 Bass kernel for NeuralGraphHidden (GNN message passing).

Math (per molecule b, atom a):
    deg[b,a]    = #valid edges (edges[b,a,:] != -1)
    summed_atom = atoms[b,a] + sum_s atoms[b, edges[b,a,s]]          (64)
    bond_sum    = sum_s bonds[b,a,s]                                  (8)
    x           = concat(summed_atom, bond_sum)                      (72)
    out[b,a]    = relu(x @ Ws[deg] + bs[deg])  if deg <= 5 else 0   (128)

Design (feature-major layout, everything folds into PE accumulation):
  * Host does all *layout* work (degree-sort permutation, neighbour row
    expansion via np.take, transposition to [feature, token] order, bf16
    packing) — pure indexed data movement.  Device does all arithmetic.
  * Feature-major: tokens are matmul rhs columns, so NO on-device
    transposes.  out^T[conv, tok] = W_d^T @ x^T with W_d the stationary
    operand, N=512 moving tiles.
  * Bond features are shipped raw (48 = 6 slots x 8); the bond-slot sum
    folds into the matmul via W rows 64:112 = tile(Wb, 6).
  * Neighbour atom rows (degree-DESC sorted, slot-s list is a prefix)
    are packed two 64-row slabs per 128 partitions.  The slab summation
    folds into the SAME PSUM accumulation via duplicated atom weights:
      [top; bottom] @ [Wa_d; Wa_d] == (top + bottom) @ Wa_d
    (DVE can't add across partition halves — lanes are partition-locked
    — but the PE contraction dimension can.)  Odd slab counts fold the
    last slab 1536/1024 at a 512-aligned boundary; its chunks use K=64
    half-array matmuls at base partition 0 (top) / 64 (bottom) so every
    matmul writes a full PSUM bank (partial-range accumulate crashes HW).
  * Bias + relu + f32->bf16 fold into one ScalarE activation per chunk
    (bias is a per-partition [128,1] AP in the conv-major layout).
  * All per-group inputs ship in 4 host-packed ~2MB superblocks ([128,W]
    exact SBUF images) — big transfers amortize the HWDGE per-transfer
    cost (~1MB knee) and pipeline with compute group by group.
  * Per-degree groups padded to 2560 slots so all 8 cores share one
    SPMD program; host unpermutes the sorted output (deg-6 rows zero).
"""

import sys

sys.path.insert(0, "/opt/trn_rl_repo")

import numpy as np
import ml_dtypes

from contextlib import ExitStack

import concourse.bacc as bacc
import concourse.tile as tile
from concourse import mybir
from concourse.bass_utils import run_bass_kernel_spmd

# Problem shapes (hardcoded per the harness contract).
B, A, D = 1024, 128, 6
F_ATOM, F_BOND, CONV = 64, 8, 128
FAN = F_ATOM + D * F_BOND               # 112 features per packed column
NCORES = 8
BS = B // NCORES          # molecules per core = 128
T = BS * A                # tokens per core = 16384
GROUP_PAD = 2560                        # per-degree group size (static)
NSORT = D * GROUP_PAD                   # 15360 sorted slots
CHUNK = 512                             # matmul moving-tile width
NCHUNK = GROUP_PAD // CHUNK             # 5 chunks per degree group
# neighbour block for degree d: floor(d/2) full pair sections + for odd
# d a 1536/1024-folded tail section (512-aligned: tokens 0:1536 on
# partitions 0:64, tokens 1536:2560 on 64:128)
TAILW = 3 * CHUNK                       # 1536 tail-section columns
NPAIR = [d // 2 for d in range(D)]
NODD = [d % 2 for d in range(D)]
NWIDTH = [NPAIR[d] * GROUP_PAD + NODD[d] * TAILW for d in range(D)]

# superblock packing: one DMA transfer per degree group (xg+ng merged,
# 0.66-2.36 MB each — above the ~1MB HWDGE efficiency knee); the weight
# blob (WBW cols) rides at the front of the first block.  Load/compute
# order is DESCENDING degree: the big blocks stream while early groups
# compute, and the group left after the last load is the smallest.
WBW = D * 2 * CONV                      # 1536 weight-blob columns
DORDER = [0, 1, 2, 3, 4, 5]
SBMAP = [[d] for d in DORDER]
XOFF, NOFF2, SBW, SBIDX = {}, {}, [], {}
for _b, _ds in enumerate(SBMAP):
    _w = WBW if _b == 0 else 0
    for _d in _ds:
        SBIDX[_d] = _b
        XOFF[_d] = _w
        _w += GROUP_PAD
        NOFF2[_d] = _w
        _w += NWIDTH[_d]
    SBW.append(_w)
NWARM = 18                              # PE warmup matmuls (HAM pre-warm)

_f32 = mybir.dt.float32
_bf16 = mybir.dt.bfloat16

_cached = {}


def build_program():
    """Build the (static) per-core Bass/Tile program."""
    nc = bacc.Bacc("TRN2", target_bir_lowering=False, debug=False)

    sbs = [nc.dram_tensor(f"sb{b}", [128, SBW[b]], _bf16,
                          kind="ExternalInput")
           for b in range(len(SBMAP) - 1)]
    bsrow = nc.dram_tensor("bsrow", [CONV, D], _f32, kind="ExternalInput")
    osort = nc.dram_tensor("osort", [CONV, NSORT], _bf16,
                           kind="ExternalOutput")

    with tile.TileContext(nc) as tc, ExitStack() as ctx:
        const_pool = ctx.enter_context(tc.tile_pool(name="const", bufs=1))
        in_pool = ctx.enter_context(tc.tile_pool(name="in", bufs=1))
        o_pool = ctx.enter_context(tc.tile_pool(name="o", bufs=1))
        ps_pool = ctx.enter_context(tc.tile_pool(name="ps", bufs=7,
                                                 space="PSUM"))
        psw_pool = ctx.enter_context(tc.tile_pool(name="psw", bufs=1,
                                                  space="PSUM"))

        # PE warmup: dummy matmuls on a zeroed scratch tile keep the PE
        # HAM clock-gate at 2.4 GHz while the first superblocks stream in
        wsrc = const_pool.tile([FAN, CHUNK], _bf16, tag="wsrc")
        nc.vector.memset(wsrc[:], 0.0)
        psw = psw_pool.tile([CONV, CHUNK], _f32, tag="psw")
        for _ in range(NWARM):
            nc.tensor.matmul(out=psw[:], lhsT=wsrc[:, 0:CONV], rhs=wsrc[:],
                             start=True, stop=True)

        bias_t = const_pool.tile([CONV, D], _f32, tag="bias")
        nc.sync.dma_start(out=bias_t[:], in_=bsrow[:])

        # d5's block ships as two column-range halves (chunks 0-2 | 3-4)
        # so its compute starts while the second half streams; loads
        # alternate between the two HWDGE rings (sync / scalar) so one
        # ring's inter-transfer ramp hides under the other's stream.
        sb5a = nc.dram_tensor("sb5a", [128, 6144], _bf16,
                              kind="ExternalInput")
        sb5b = nc.dram_tensor("sb5b", [128, 3072], _bf16,
                              kind="ExternalInput")
        sbt = []
        for b in range(len(SBMAP) - 1):
            t = in_pool.tile([128, SBW[b]], _bf16, tag=f"sb{b}")
            eng = nc.sync if b % 2 == 0 else nc.scalar
            eng.dma_start(out=t[:], in_=sbs[b][:])
            sbt.append(t)
        t5a = in_pool.tile([128, 6144], _bf16, tag="sb5a")
        nc.scalar.dma_start(out=t5a[:], in_=sb5a[:])
        t5b = in_pool.tile([128, 3072], _bf16, tag="sb5b")
        nc.sync.dma_start(out=t5b[:], in_=sb5b[:])
        wb = sbt[0]                        # weight blob at front of block 0

        for d in DORDER:
            g = D - 1 - d                  # block index (DESC-sorted layout)
            if d == DORDER[-1]:
                # hold the PE clock-gate warm through the load-starved
                # stretch before the last (largest) group's block lands
                for _ in range(14):
                    nc.tensor.matmul(out=psw[:], lhsT=wsrc[:, 0:CONV],
                                     rhs=wsrc[:], start=True, stop=True)

            wt = wb[0:FAN, d * 2 * CONV:d * 2 * CONV + CONV]
            wa2 = wb[:, d * 2 * CONV + CONV:(d + 1) * 2 * CONV]
            og = o_pool.tile([CONV, GROUP_PAD], _bf16, tag=f"og{d}")
            for j in range(NCHUNK):
                c0, c1 = j * CHUNK, (j + 1) * CHUNK
                # (lhsT, rhs) accumulating into this chunk's bank
                if d == 5:
                    # split block: A = chunks 0-2 (sections of 1536 cols
                    # + whole folded tail), B = chunks 3-4 (1024 cols)
                    if j < 3:
                        mms = [(wt, t5a[0:FAN, c0:c1])]
                        for p in range(2):
                            o0 = TAILW * (p + 1) + c0
                            mms.append((wa2, t5a[:, o0:o0 + CHUNK]))
                        mms.append(
                            (wa2[0:64, :],
                             t5a[0:64, 3 * TAILW + c0:3 * TAILW + c1]))
                    else:
                        cb = c0 - TAILW
                        mms = [(wt, t5b[0:FAN, cb:cb + CHUNK])]
                        for p in range(2):
                            o0 = 1024 * (p + 1) + cb
                            mms.append((wa2, t5b[:, o0:o0 + CHUNK]))
                        mms.append(
                            (wa2[64:128, :],
                             t5a[64:128, 3 * TAILW + cb:
                                 3 * TAILW + cb + CHUNK]))
                else:
                    sb = sbt[SBIDX[d]]
                    xg = sb[0:FAN, XOFF[d]:XOFF[d] + GROUP_PAD]
                    no = NOFF2[d]
                    mms = [(wt, xg[:, c0:c1])]
                    for p in range(NPAIR[d]):
                        mms.append(
                            (wa2, sb[:, no + p * GROUP_PAD + c0:
                                     no + p * GROUP_PAD + c1]))
                    if NODD[d]:
                        toff = no + NPAIR[d] * GROUP_PAD
                        if c1 <= TAILW:    # tokens 0:1536 on top half
                            mms.append(
                                (wa2[0:64, :],
                                 sb[0:64, toff + c0:toff + c1]))
                        else:              # tokens 1536:2560 on bottom
                            mms.append(
                                (wa2[64:128, :],
                                 sb[64:128, toff + c0 - TAILW:
                                    toff + c1 - TAILW]))

                ps = ps_pool.tile([CONV, CHUNK], _f32, tag="ps")
                for i, (lhsT, rhs) in enumerate(mms):
                    nc.tensor.matmul(
                        out=ps[:], lhsT=lhsT, rhs=rhs,
                        start=(i == 0), stop=(i == len(mms) - 1))
                nc.scalar.activation(
                    og[:, c0:c1], ps[:],
                    mybir.ActivationFunctionType.Relu,
                    bias=bias_t[:, d:d + 1])
                if j == 2:                 # first 1536 cols ready — store
                    nc.scalar.dma_start(
                        out=osort[:, g * GROUP_PAD:g * GROUP_PAD + TAILW],
                        in_=og[:, 0:TAILW])
            nc.scalar.dma_start(
                out=osort[:, g * GROUP_PAD + TAILW:(g + 1) * GROUP_PAD],
                in_=og[:, TAILW:GROUP_PAD])

    nc.compile()
    return nc


def _get_program():
    if "nc" not in _cached:
        _cached["nc"] = build_program()
    return _cached["nc"]


def prep_core_inputs(atoms_s, bonds_s, edges_s, wblob_np, bsrow_np):
    """Host-side layout/index prep for one core's shard (numpy only)."""
    deg = (edges_s != -1).sum(axis=-1).reshape(-1)            # [T] natural
    slot_tok = np.full(NSORT, -1, np.int64)   # sorted slot -> natural token
    for d in range(D):
        toks = np.nonzero(deg == d)[0]
        n = len(toks)
        assert n <= GROUP_PAD, f"degree-{d} group has {n} > {GROUP_PAD}"
        base = (D - 1 - d) * GROUP_PAD
        slot_tok[base:base + n] = toks

    flat_a = atoms_s.reshape(T, F_ATOM).astype(ml_dtypes.bfloat16)
    flat_b = bonds_s.reshape(T, D * F_BOND).astype(ml_dtypes.bfloat16)
    valid = slot_tok >= 0
    safe = np.maximum(slot_tok, 0)
    x = np.concatenate(
        [np.where(valid[:, None], flat_a[safe], ml_dtypes.bfloat16(0)),
         np.where(valid[:, None], flat_b[safe], ml_dtypes.bfloat16(0))],
        axis=1)                                               # [NSORT, 112]

    eflat = edges_s.reshape(T, D)
    bcol = (np.arange(T) // A) * A                            # molecule base
    sblocks = [np.zeros((128, w), ml_dtypes.bfloat16) for w in SBW]
    sblocks[0][:, :WBW] = wblob_np
    for d in range(D):
        g = D - 1 - d
        blk = sblocks[SBIDX[d]]
        # x tile, feature-major (rows 112:128 stay zero)
        blk[0:FAN, XOFF[d]:XOFF[d] + GROUP_PAD] = \
            x[g * GROUP_PAD:(g + 1) * GROUP_PAD].T
        if d == 0:
            continue
        slots = slot_tok[g * GROUP_PAD:(g + 1) * GROUP_PAD]
        sv = np.maximum(slots, 0)
        slabs = []
        for s in range(d):
            e = np.where(slots >= 0, eflat[sv, s], -1)
            nat = np.maximum(bcol[sv] + e, 0)
            rows = np.where((e >= 0)[:, None], flat_a[nat],
                            ml_dtypes.bfloat16(0))            # [2560, 64]
            slabs.append(np.ascontiguousarray(rows.T))        # [64, 2560]
        no = NOFF2[d]
        for p in range(NPAIR[d]):
            blk[0:64, no + p * GROUP_PAD:no + (p + 1) * GROUP_PAD] = \
                slabs[2 * p]
            blk[64:128, no + p * GROUP_PAD:no + (p + 1) * GROUP_PAD] = \
                slabs[2 * p + 1]
        if NODD[d]:
            toff = no + NPAIR[d] * GROUP_PAD
            blk[0:64, toff:toff + TAILW] = slabs[d - 1][:, :TAILW]
            blk[64:128, toff:toff + GROUP_PAD - TAILW] = \
                slabs[d - 1][:, TAILW:]

    m = {f"sb{b}": np.ascontiguousarray(sblocks[b])
         for b in range(len(SBW) - 1)}
    # d5's block re-cut into chunk-range halves (A: token cols 0:1536 of
    # xg/p0/p1 + the whole folded tail; B: token cols 1536:2560)
    b5 = sblocks[len(SBW) - 1]
    a = np.zeros((128, 6144), ml_dtypes.bfloat16)
    bb = np.zeros((128, 3072), ml_dtypes.bfloat16)
    for s, off in enumerate([0, 2560, 5120]):      # xg, pair0, pair1
        a[:, s * TAILW:(s + 1) * TAILW] = b5[:, off:off + TAILW]
        bb[:, s * 1024:(s + 1) * 1024] = b5[:, off + TAILW:off + GROUP_PAD]
    a[:, 3 * TAILW:4 * TAILW] = b5[:, 7680:9216]   # folded tail section
    m["sb5a"] = np.ascontiguousarray(a)
    m["sb5b"] = np.ascontiguousarray(bb)
    m["bsrow"] = bsrow_np
    return m, slot_tok


def kernel(atoms, bonds, edges, Ws, bs, trace=False):
    atoms = np.asarray(atoms)
    bonds = np.asarray(bonds)
    edges = np.asarray(edges)
    Ws = np.asarray(Ws)
    bs = np.asarray(bs)

    # Wfull rows = [Wa (64) | tile(Wb, 6) (48)]; bias via ACT bias AP
    wblob_np = np.zeros((128, D * 2 * CONV), np.float32)
    for d in range(D):
        wblob_np[:F_ATOM, d * 2 * CONV:d * 2 * CONV + CONV] = Ws[d, :F_ATOM]
        wblob_np[F_ATOM:FAN, d * 2 * CONV:d * 2 * CONV + CONV] = \
            np.tile(Ws[d, F_ATOM:], (D, 1))
        # duplicated atom weights for the partition-fold matmuls
        wblob_np[0:64, d * 2 * CONV + CONV:(d + 1) * 2 * CONV] = \
            Ws[d, :F_ATOM]
        wblob_np[64:128, d * 2 * CONV + CONV:(d + 1) * 2 * CONV] = \
            Ws[d, :F_ATOM]
    wblob_np = wblob_np.astype(ml_dtypes.bfloat16)
    bsrow_np = np.ascontiguousarray(bs.T.astype(np.float32))  # [CONV, D]

    in_maps, slot_toks = [], []
    for c in range(NCORES):
        sl = slice(c * BS, (c + 1) * BS)
        m, st = prep_core_inputs(atoms[sl], bonds[sl], edges[sl],
                                 wblob_np, bsrow_np)
        in_maps.append(m)
        slot_toks.append(st)

    nc = _get_program()
    res = run_bass_kernel_spmd(nc, in_maps, core_ids=list(range(NCORES)),
                               trace=trace)
    kernel.last_results = res

    out = np.zeros((B, A, CONV), np.float32)
    for c in range(NCORES):
        osort = res.results[c]["osort"].view(ml_dtypes.bfloat16)
        osort = osort.reshape(CONV, NSORT)                    # conv-major
        st = slot_toks[c]
        real = st >= 0
        shard = out[c * BS:(c + 1) * BS].reshape(T, CONV)
        shard[st[real]] = osort[:, real].T.astype(np.float32)
    return out
